# revision 8
# baseline (speedup 1.0000x reference)
"""SE(3) attention block (GNN message passing) on 8 Trainium2 NeuronCores.

Strategy (slot format, v4)
--------------------------
Nodes are sorted by in-degree (host) and cut into tiles of 128 nodes.
Tiles are grouped into batches of 8*b tiles (b per core, SPMD-identical
shapes) padded to the batch max degree S; degree sorting keeps slot
padding at ~2-4% of E.

Each node-row owns its incoming edges as "slots" 0..S-1, so the segment
softmax and the weighted aggregation become *free-axis* operations on the
node-partitioned tile.  v4 rebalances the engines so each of DVE / ACT /
PE / DMA carries ~one M-sized pass (M = E*128/8 elems per core):

  DVE : prodT = kT*qT (fp16 2x, q broadcast over slots)   [M]
        evex  = v*exw (fp16 2x)                            [M]
  PE  : scores = per-slot (prodT)^T @ head-mask -> PSUM
        ssum   = identity-accumulate of narrow ex (wrapped out AP)
        sc    -= sqrt(NF)*ln(ssum)  (-sqrt(NF)*I @ ln(ssum), rhs wrap)
        agg    = identity-accumulate of evex slots (wrapped out AP)
  ACT : ex  = exp(sc/sqrt(NF)) narrow; lnss = log(ssum)
        exw = exp(sc'/sqrt(NF)) widened 16x  (the 1x j-broadcast pass
        lives here by design -- DVE broadcast-innermost would be 1x too)
        out = copy(agg) PSUM->SBUF fp16
  DMA : kT ships as fp8-e3m4 and upcasts to fp16 in-flight (SWDGE cast,
        gpsimd ring); v ships fp16 on the HWDGE sync ring so the two big
        streams ride different descriptor paths.

Because ln(ssum) is subtracted inside the PSUM scores, the exp-widen
directly emits normalized softmax weights: no reciprocal, no final
normalize pass.  Padding slots carry k_pad = -15.5*sign(q) (exact in
e3m4) so every head scores <= ~-5 std below real scores and v_pad = 0;
pad weights underflow fp16 to ~0 and contribute nothing.
"""

import math
import numpy as np

# ---------------------------------------------------------------- constants
N_CORES = 8
P = 128                 # partitions / nodes per tile
H = 8                   # heads
NF = 128                # features per edge (32*4)
HS = NF // H            # head size (16)
INV_SQRT_NF = 1.0 / math.sqrt(NF)
SQRT_NF = math.sqrt(NF)
K_PAD = 15.5            # pad key magnitude (exact in fp8-e3m4)
AGW = NF + H            # agg PSUM block per tile: 128 feat + 8 ssum cols
MAX_BATCH_SLOTS = 44    # b*S cap (PSUM bank + SBUF budget)
MAX_B = 6               # tiles per core per batch cap

_CACHE = {}
LAST_RESULTS = None     # BassKernelResults of the most recent run (for test.py)


# ---------------------------------------------------------------- device IR
def build_nc(batch_prof):
    """Per-core Bass/Tile program; identical on all 8 cores (SPMD).

    batch_prof: tuple of (S, b) per batch.
    """
    from contextlib import ExitStack

    import concourse.bacc as bacc
    import concourse.mybir as mybir
    from concourse.tile import TileContext

    f32 = mybir.dt.float32
    f16 = mybir.dt.float16
    f8 = mybir.dt.float8e3
    W = int(sum(S * b for S, b in batch_prof)) * P
    Wq = int(sum(b for S, b in batch_prof)) * P

    nc = bacc.Bacc("TRN2", target_bir_lowering=False, debug=False)
    kT_d = nc.dram_tensor("kT", [P, W], f8, kind="ExternalInput")
    v_d = nc.dram_tensor("v", [P, W], f16, kind="ExternalInput")
    qT_d = nc.dram_tensor("qT", [P, Wq], f16, kind="ExternalInput")
    hm_d = nc.dram_tensor("hm", [P, H], f16, kind="ExternalInput")
    id_d = nc.dram_tensor("ident", [P, P], f16, kind="ExternalInput")
    ids_d = nc.dram_tensor("identS", [P, P], f16, kind="ExternalInput")
    out_d = nc.dram_tensor("out", [P, Wq], f16, kind="ExternalOutput")

    with TileContext(nc) as tc, ExitStack() as ctx:
        singles = ctx.enter_context(tc.tile_pool(name="singles", bufs=1))
        inp = ctx.enter_context(tc.tile_pool(name="inp", bufs=3))
        mid = ctx.enter_context(tc.tile_pool(name="mid", bufs=2))
        sml = ctx.enter_context(tc.tile_pool(name="sml", bufs=3))
        ps = ctx.enter_context(tc.tile_pool(name="ps", bufs=2, space="PSUM"))
        psa = ctx.enter_context(tc.tile_pool(name="psa", bufs=2, space="PSUM"))

        hm = singles.tile([P, H], f16)
        nc.sync.dma_start(out=hm[:], in_=hm_d[:, :])
        ident = singles.tile([P, P], f16)
        nc.sync.dma_start(out=ident[:], in_=id_d[:, :])
        identS = singles.tile([P, P], f16)
        nc.sync.dma_start(out=identS[:], in_=ids_d[:, :])

        offs = []
        off = offq = 0
        for S, b in batch_prof:
            offs.append((off, offq))
            off += int(S) * int(b) * P
            offq += int(b) * P

        def stage_a(i):
            """DMA in, prodT, score matmuls, narrow exp."""
            S, b = int(batch_prof[i][0]), int(batch_prof[i][1])
            off, offq = offs[i]
            Wt = b * S * P
            kT = inp.tile([P, Wt], f16, tag="kT")
            nc.gpsimd.dma_start(out=kT[:], in_=kT_d[:, off:off + Wt])
            v = inp.tile([P, Wt], f16, tag="v")
            nc.sync.dma_start(out=v[:], in_=v_d[:, off:off + Wt])
            qT = sml.tile([P, b * P], f16, tag="qT")
            nc.gpsimd.dma_start(out=qT[:], in_=qT_d[:, offq:offq + b * P])

            prodT = mid.tile([P, Wt], f16, tag="prodT")
            nc.vector.tensor_tensor(
                out=prodT[:].rearrange("p (t s n) -> p t s n", t=b, s=S),
                in0=kT[:].rearrange("p (t s n) -> p t s n", t=b, s=S),
                in1=qT[:].rearrange("p (t n) -> p t n", t=b)
                    .unsqueeze(2).broadcast_to([P, b, S, P]),
                op=mybir.AluOpType.mult)

            # one open accumulation group per sc tile: ts=0 arms the whole
            # PSUM bank (pending-zero), later slots zero+write their own
            # bytes, and stage_b's ln(ssum) correction closes the group
            sc = ps.tile([P, b * S * H], f32, tag="sc")
            for ts in range(b * S):
                nc.tensor.matmul(
                    out=sc[:, ts * H:(ts + 1) * H],
                    lhsT=prodT[:, ts * P:(ts + 1) * P], rhs=hm[:],
                    start=(ts == 0), stop=False, skip_group_check=True)

            # narrow ex[p, (t, h, s)] = exp(sc / sqrt(NF))
            ex = sml.tile([P, b * S * H], f16, tag="ex")
            nc.scalar.activation(
                out=ex[:].rearrange("p (t h s) -> p t h s", t=b, h=H),
                in_=sc[:].rearrange("p (t s h) -> p t h s", t=b, s=S),
                func=mybir.ActivationFunctionType.Exp, scale=INV_SQRT_NF)
            return (S, b, offq, v, sc, ex)

        def stage_b(state):
            """ssum, log-correct, widen, evex, fold, out."""
            S, b, offq, v, sc, ex = state

            # ssum[p, (t, h)] = sum_s ex  (f32)
            ssum = sml.tile([P, b * H], f32, tag="ssum")
            nc.vector.tensor_reduce(
                out=ssum[:],
                in_=ex[:].rearrange("p (th s) -> p th s", s=S),
                axis=mybir.AxisListType.X, op=mybir.AluOpType.add)
            lnss = sml.tile([P, b * H], f16, tag="lnss")
            nc.scalar.activation(
                out=lnss[:], in_=ssum[:],
                func=mybir.ActivationFunctionType.Ln, scale=1.0)

            # sc -= sqrt(NF) * ln(ssum)  (one PE matmul, rhs broadcast over
            # slots; closes the sc accumulation group)
            nc.tensor.matmul(
                out=sc[:],
                lhsT=identS[:],
                rhs=lnss[:].rearrange("p (t h) -> p t h", t=b)
                    .unsqueeze(2).broadcast_to([P, b, S, H]),
                start=False, stop=True, skip_group_check=True)

            # exw[p, (t, s, h, j)] = exp(sc' / sqrt(NF)): normalized softmax
            # weights widened 16x (the per-j broadcast pass lives on ACT)
            exw = mid.tile([P, b * S * P], f16, tag="exw")
            for t in range(b):
                nc.scalar.activation(
                    out=exw[:, t * S * P:(t + 1) * S * P]
                        .rearrange("p (s h j) -> p s h j", s=S, h=H),
                    in_=sc[:, t * S * H:(t + 1) * S * H]
                        .rearrange("p (s h) -> p s h", s=S)
                        .to_broadcast([P, S, H, HS]),
                    func=mybir.ActivationFunctionType.Exp, scale=INV_SQRT_NF)

            evex = mid.tile([P, b * S * P], f16, tag="evex")
            nc.vector.tensor_tensor(
                out=evex[:], in0=v[:], in1=exw[:], op=mybir.AluOpType.mult)

            # weighted scatter-sum: per-slot identity-accumulate on PE
            # (LDW overlaps the previous matmul's stream, so each slot costs
            # ~max(128, cols) cycles -- same as a fused wrap would)
            agg = psa.tile([P, b * NF], f32, tag="agg")
            for t in range(b):
                for s in range(S):
                    nc.tensor.matmul(
                        out=agg[:, t * NF:(t + 1) * NF],
                        lhsT=ident[:],
                        rhs=evex[:, (t * S + s) * P:(t * S + s + 1) * P],
                        start=(s == 0), stop=(s == S - 1))

            outb = sml.tile([P, b * P], f16, tag="outb")
            nc.scalar.copy(out=outb[:], in_=agg[:])
            nc.sync.dma_start(out=out_d[:, offq:offq + b * P], in_=outb[:])

        # software-pipelined emission: batch i+1's front half goes ahead of
        # batch i's back half so the in-order DVE queue never stalls on ACT
        nb = len(batch_prof)
        pend = stage_a(0)
        for i in range(1, nb):
            nxt_state = stage_a(i)
            stage_b(pend)
            pend = nxt_state
        stage_b(pend)
    nc.compile()
    return nc


# ------------------------------------------------------------ host plumbing
def _plan(edge_index, n_nodes):
    """Degree-sorted batched tile plan shared by all cores."""
    dst = np.asarray(edge_index)[1].astype(np.int64).ravel()
    n_edges = dst.shape[0]
    counts = np.bincount(dst, minlength=n_nodes)
    order_e = np.argsort(dst, kind="stable")
    cum = np.zeros(n_nodes + 1, np.int64)
    cum[1:] = np.cumsum(counts)
    nperm = np.argsort(-counts, kind="stable")

    n_tiles = -(-n_nodes // P)
    deg_desc = np.zeros(n_tiles * P, np.int64)
    deg_desc[:n_nodes] = counts[nperm]

    batches = []            # (S, b, tile_start)
    t = 0
    while t < n_tiles:
        S = max(int(deg_desc[t * P]), 4)
        rem_groups = -(-(n_tiles - t) // N_CORES)
        b = max(1, min(MAX_B, MAX_BATCH_SLOTS // S, rem_groups))
        batches.append((S, b, t))
        t += N_CORES * b
    batches = batches[-1:] + batches[:-1]   # smallest batch first: fast ramp

    total_tiles = sum(N_CORES * b for S, b, _ in batches)
    rnode = np.full(total_tiles * P, -1, np.int64)
    rnode[:n_nodes] = nperm
    return dict(counts=counts, order_e=order_e, cum=cum, rnode=rnode,
                batches=batches, n_edges=n_edges, n_nodes=n_nodes)


def _prep_inputs(value, key, query_0, query_1, plan):
    import ml_dtypes
    f16 = np.float16
    f8 = ml_dtypes.float8_e3m4

    batches = plan["batches"]
    rnode = plan["rnode"]
    counts, order_e, cum = plan["counts"], plan["order_e"], plan["cum"]
    n_edges = plan["n_edges"]
    n_nodes = plan["n_nodes"]

    key_f = np.asarray(key, dtype=np.float32).reshape(n_edges, NF)
    val_f = np.asarray(value, dtype=np.float32).reshape(n_edges, NF)
    q_cat = np.concatenate(
        [np.asarray(query_0, np.float32), np.asarray(query_1, np.float32)],
        axis=-1).reshape(n_nodes, NF)
    # pad keys: every head must score far below real scores; with
    # k_pad = -K_PAD*sign(q) each head's pad score is -K_PAD*sum|q_j|
    kpad = (-K_PAD * np.sign(q_cat)).astype(np.float32)

    W = sum(S * b for S, b, _ in batches) * P
    Wq = sum(b for S, b, _ in batches) * P
    kT_all = np.empty((N_CORES, P, W), f8)
    v_all = np.empty((N_CORES, P, W), f16)
    qT_all = np.empty((N_CORES, P, Wq), f16)

    off = 0
    offq = 0
    for S, b, t0 in batches:
        nb = N_CORES * b * P
        rows = rnode[t0 * P:t0 * P + nb]
        valid_r = rows >= 0
        rr = np.where(valid_r, rows, 0)
        deg = np.where(valid_r, counts[rr], 0)
        start = cum[rr]
        sl = np.arange(S)
        eix = start[:, None] + sl[None, :]
        vmask = sl[None, :] < deg[:, None]
        eid = order_e[np.clip(eix, 0, n_edges - 1)]

        kg = key_f[eid]                        # [8*b*128, S, NF]
        kp = kpad[rr] * valid_r[:, None]
        kg = np.where(vmask[..., None], kg, kp[:, None, :])
        vg = val_f[eid]
        vg[~vmask] = 0.0
        qg = q_cat[rr] * valid_r[:, None]

        # [core, t, n, S, NF]
        kg = kg.reshape(N_CORES, b, P, S, NF)
        vg = vg.reshape(N_CORES, b, P, S, NF)
        qg = qg.reshape(N_CORES, b, P, NF)
        # kT: [f, (t, s, n)]
        kT_all[:, :, off:off + b * S * P] = kg.transpose(0, 4, 1, 3, 2).reshape(
            N_CORES, NF, b * S * P).astype(f8)
        # v: [n, (t, s, f)]
        v_all[:, :, off:off + b * S * P] = vg.transpose(0, 2, 1, 3, 4).reshape(
            N_CORES, P, b * S * NF).astype(f16)
        # qT: [f, (t, n)]
        qT_all[:, :, offq:offq + b * P] = qg.transpose(0, 3, 1, 2).reshape(
            N_CORES, NF, b * P).astype(f16)
        off += b * S * P
        offq += b * P

    hm = np.zeros((NF, H), np.float32)
    for h in range(H):
        hm[h * HS:(h + 1) * HS, h] = 1.0
    hm = hm.astype(f16)
    ident = np.eye(P, dtype=np.float32).astype(f16)
    identS = (-SQRT_NF * np.eye(P, dtype=np.float32)).astype(f16)

    in_maps = []
    for c in range(N_CORES):
        in_maps.append({
            "kT": kT_all[c], "v": v_all[c], "qT": qT_all[c], "hm": hm,
            "ident": ident, "identS": identS,
        })
    return in_maps


def _assemble(results, plan):
    batches = plan["batches"]
    n_nodes = plan["n_nodes"]
    rnode = plan["rnode"]
    out = np.zeros((n_nodes, NF), np.float32)
    for c in range(N_CORES):
        arr = np.asarray(results[c]["out"], dtype=np.float32)  # [P, Wq]
        offq = 0
        for S, b, t0 in batches:
            blk = arr[:, offq:offq + b * P].reshape(P, b, NF)
            rows = rnode[(t0 + c * b) * P:(t0 + (c + 1) * b) * P].reshape(
                b, P)
            for t in range(b):
                idx = rows[t]
                m = idx >= 0
                out[idx[m]] = blk[m, t]
            offq += b * P
    return out.reshape(n_nodes, NF // 4, 4)


def _get_nc(batch_prof):
    key = tuple(batch_prof)
    if key not in _CACHE:
        _CACHE[key] = build_nc(batch_prof)
    return _CACHE[key]


def _run(inputs, trace=False, **spmd_kwargs):
    global LAST_RESULTS
    from concourse.bass_utils import run_bass_kernel_spmd

    n_nodes = np.asarray(inputs["query_0"]).shape[0]
    plan = _plan(inputs["edge_index"], n_nodes)
    batch_prof = tuple((int(S), int(b)) for S, b, _ in plan["batches"])
    nc = _get_nc(batch_prof)
    in_maps = _prep_inputs(
        inputs["value"], inputs["key"], inputs["query_0"], inputs["query_1"],
        plan)
    res = run_bass_kernel_spmd(
        nc, in_maps, list(range(N_CORES)), trace=trace, **spmd_kwargs)
    LAST_RESULTS = res
    return _assemble(res.results, plan)


def kernel(value, key, query_0, query_1, edge_index):
    return _run({
        "value": value, "key": key, "query_0": query_0,
        "query_1": query_1, "edge_index": edge_index,
    })


# revision 14
# speedup vs baseline: 1.2480x; 1.2480x over previous
"""SE(3) attention block (GNN message passing) on 8 Trainium2 NeuronCores.

Strategy (slot format, v4)
--------------------------
Nodes are sorted by in-degree (host) and cut into tiles of 128 nodes.
Tiles are grouped into batches of 8*b tiles (b per core, SPMD-identical
shapes) padded to the batch max degree S; degree sorting keeps slot
padding at ~2-4% of E.

Each node-row owns its incoming edges as "slots" 0..S-1, so the segment
softmax and the weighted aggregation become *free-axis* operations on the
node-partitioned tile.  v4 rebalances the engines so each of DVE / ACT /
PE / DMA carries ~one M-sized pass (M = E*128/8 elems per core):

  DVE : prodT = kT*qT (fp16 2x, q broadcast over slots)   [M]
        evex  = v*exw (fp16 2x)                            [M]
  PE  : scores = per-slot (prodT)^T @ head-mask -> PSUM
        ssum   = identity-accumulate of narrow ex (wrapped out AP)
        sc    -= sqrt(NF)*ln(ssum)  (-sqrt(NF)*I @ ln(ssum), rhs wrap)
        agg    = identity-accumulate of evex slots (wrapped out AP)
  ACT : ex  = exp(sc/sqrt(NF)) narrow; lnss = log(ssum)
        exw = exp(sc'/sqrt(NF)) widened 16x  (the 1x j-broadcast pass
        lives here by design -- DVE broadcast-innermost would be 1x too)
        out = copy(agg) PSUM->SBUF fp16
  DMA : kT ships as fp8-e3m4 and upcasts to fp16 in-flight (SWDGE cast,
        gpsimd ring); v ships fp16 on the HWDGE sync ring so the two big
        streams ride different descriptor paths.

Because ln(ssum) is subtracted inside the PSUM scores, the exp-widen
directly emits normalized softmax weights: no reciprocal, no final
normalize pass.  Padding slots carry k_pad = -15.5*sign(q) (exact in
e3m4) so every head scores <= ~-5 std below real scores and v_pad = 0;
pad weights underflow fp16 to ~0 and contribute nothing.
"""

import math
import numpy as np

# ---------------------------------------------------------------- constants
N_CORES = 8
P = 128                 # partitions / nodes per tile
H = 8                   # heads
NF = 128                # features per edge (32*4)
HS = NF // H            # head size (16)
INV_SQRT_NF = 1.0 / math.sqrt(NF)
SQRT_NF = math.sqrt(NF)
K_PAD = 15.5            # pad key magnitude (exact in fp8-e3m4)
AGW = NF + H            # agg PSUM block per tile: 128 feat + 8 ssum cols
MAX_BATCH_SLOTS = 44    # b*S cap (PSUM bank + SBUF budget)
MAX_B = 6               # tiles per core per batch cap

_CACHE = {}
LAST_RESULTS = None     # BassKernelResults of the most recent run (for test.py)


# ---------------------------------------------------------------- device IR
def build_nc(batch_prof):
    """Per-core Bass/Tile program; identical on all 8 cores (SPMD).

    batch_prof: tuple of (S, b) per batch.
    """
    from contextlib import ExitStack

    import concourse.bacc as bacc
    import concourse.mybir as mybir
    from concourse.tile import TileContext

    f32 = mybir.dt.float32
    f16 = mybir.dt.float16
    f8 = mybir.dt.float8e3
    W = int(sum(S * b for S, b in batch_prof)) * P
    Wq = int(sum(b for S, b in batch_prof)) * P

    nc = bacc.Bacc("TRN2", target_bir_lowering=False, debug=False)
    f8e4 = mybir.dt.float8e4
    kT_d = nc.dram_tensor("kT", [P, W], f8, kind="ExternalInput")
    v_d = nc.dram_tensor("v", [P, W], f16, kind="ExternalInput")
    qT_d = nc.dram_tensor("qT", [P, Wq], f16, kind="ExternalInput")
    hm_d = nc.dram_tensor("hm", [P, H], f16, kind="ExternalInput")
    id8_d = nc.dram_tensor("ident8", [P, P], f8e4, kind="ExternalInput")
    out_d = nc.dram_tensor("out", [P, Wq], f16, kind="ExternalOutput")

    with TileContext(nc) as tc, ExitStack() as ctx:
        singles = ctx.enter_context(tc.tile_pool(name="singles", bufs=1))
        inp = ctx.enter_context(tc.tile_pool(name="inp", bufs=3))
        mid = ctx.enter_context(tc.tile_pool(name="mid", bufs=2))
        sml = ctx.enter_context(tc.tile_pool(name="sml", bufs=3))
        ps = ctx.enter_context(tc.tile_pool(name="ps", bufs=2, space="PSUM"))
        psa = ctx.enter_context(tc.tile_pool(name="psa", bufs=2, space="PSUM"))

        hm = singles.tile([P, H], f16)
        nc.sync.dma_start(out=hm[:], in_=hm_d[:, :])
        ident8 = singles.tile([P, P], f8e4)
        nc.sync.dma_start(out=ident8[:], in_=id8_d[:, :])

        offs = []
        off = offq = 0
        for S, b in batch_prof:
            offs.append((off, offq))
            off += int(S) * int(b) * P
            offq += int(b) * P

        def stage_a(i):
            """DMA in, prodT, score matmuls, narrow exp."""
            S, b = int(batch_prof[i][0]), int(batch_prof[i][1])
            off, offq = offs[i]
            Wt = b * S * P
            kT = inp.tile([P, Wt], f16, tag="kT")
            nc.gpsimd.dma_start(out=kT[:], in_=kT_d[:, off:off + Wt])
            v = inp.tile([P, Wt], f16, tag="v")
            nc.sync.dma_start(out=v[:], in_=v_d[:, off:off + Wt])
            qT = sml.tile([P, b * P], f16, tag="qT")
            nc.gpsimd.dma_start(out=qT[:], in_=qT_d[:, offq:offq + b * P])

            prodT = mid.tile([P, Wt], f16, tag="prodT")
            nc.vector.tensor_tensor(
                out=prodT[:].rearrange("p (t s n) -> p t s n", t=b, s=S),
                in0=kT[:].rearrange("p (t s n) -> p t s n", t=b, s=S),
                in1=qT[:].rearrange("p (t n) -> p t n", t=b)
                    .unsqueeze(2).broadcast_to([P, b, S, P]),
                op=mybir.AluOpType.mult)

            sc = ps.tile([P, b * S * H], f32, tag="sc")
            for ts in range(b * S):
                nc.tensor.matmul(
                    out=sc[:, ts * H:(ts + 1) * H],
                    lhsT=prodT[:, ts * P:(ts + 1) * P], rhs=hm[:],
                    start=True, stop=True)

            # narrow ex[p, (t, h, s)] = exp(sc / sqrt(NF))
            ex = sml.tile([P, b * S * H], f16, tag="ex")
            nc.scalar.activation(
                out=ex[:].rearrange("p (t h s) -> p t h s", t=b, h=H),
                in_=sc[:].rearrange("p (t s h) -> p t h s", t=b, s=S),
                func=mybir.ActivationFunctionType.Exp, scale=INV_SQRT_NF)
            return (S, b, offq, v, sc, ex)

        def stage_b(state):
            """ssum, log-correct, widen, evex, fold, out."""
            S, b, offq, v, sc, ex = state

            # ssum[p, (t, h)] = sum_s ex  (f32); inv = 1/ssum
            ssum = sml.tile([P, b * H], f32, tag="ssum")
            nc.vector.tensor_reduce(
                out=ssum[:],
                in_=ex[:].rearrange("p (th s) -> p th s", s=S),
                axis=mybir.AxisListType.X, op=mybir.AluOpType.add)
            inv = sml.tile([P, b * H], f32, tag="inv")
            nc.vector.reciprocal_approx_fast(out=inv[:], in_=ssum[:])

            # exw[p, (t, s, h, j)] = exp(sc / sqrt(NF)) widened 16x (the
            # per-j broadcast pass lives on ACT; Exp only -> one table load)
            exw = mid.tile([P, b * S * P], f16, tag="exw")
            for t in range(b):
                nc.scalar.activation(
                    out=exw[:, t * S * P:(t + 1) * S * P]
                        .rearrange("p (s h j) -> p s h j", s=S, h=H),
                    in_=sc[:, t * S * H:(t + 1) * S * H]
                        .rearrange("p (s h) -> p s h", s=S)
                        .to_broadcast([P, S, H, HS]),
                    func=mybir.ActivationFunctionType.Exp, scale=INV_SQRT_NF)

            evex = mid.tile([P, b * S * P], f16, tag="evex")
            nc.vector.tensor_tensor(
                out=evex[:], in0=v[:], in1=exw[:], op=mybir.AluOpType.mult)

            # weighted scatter-sum: per-slot identity-accumulate on PE
            # (fp8 identity halves the LDWEIGHTS cost per slot)
            agg = psa.tile([P, b * NF], f32, tag="agg")
            for t in range(b):
                for s in range(S):
                    nc.tensor.matmul(
                        out=agg[:, t * NF:(t + 1) * NF],
                        lhsT=ident8[:],
                        rhs=evex[:, (t * S + s) * P:(t * S + s + 1) * P],
                        start=(s == 0), stop=(s == S - 1))

            # out = agg * inv (bf16 out; agg PSUM src)
            outb = sml.tile([P, b * P], f16, tag="outb")
            nc.vector.tensor_tensor(
                out=outb[:].rearrange("p (t h j) -> p t h j", t=b, h=H),
                in0=agg[:].rearrange("p (t h j) -> p t h j", t=b, h=H),
                in1=inv[:].rearrange("p (t h) -> p t h", t=b)
                    .to_broadcast([P, b, H, HS]),
                op=mybir.AluOpType.mult)
            nc.sync.dma_start(out=out_d[:, offq:offq + b * P], in_=outb[:])

        # software-pipelined emission: batch i+1's front half goes ahead of
        # batch i's back half so the in-order DVE queue never stalls on ACT
        nb = len(batch_prof)
        pend = stage_a(0)
        for i in range(1, nb):
            nxt_state = stage_a(i)
            stage_b(pend)
            pend = nxt_state
        stage_b(pend)
    nc.compile()
    return nc


# ------------------------------------------------------------ host plumbing
def _plan(edge_index, n_nodes):
    """Degree-sorted batched tile plan shared by all cores."""
    dst = np.asarray(edge_index)[1].astype(np.int64).ravel()
    n_edges = dst.shape[0]
    counts = np.bincount(dst, minlength=n_nodes)
    order_e = np.argsort(dst, kind="stable")
    cum = np.zeros(n_nodes + 1, np.int64)
    cum[1:] = np.cumsum(counts)
    nperm = np.argsort(-counts, kind="stable")

    n_tiles = -(-n_nodes // P)
    deg_desc = np.zeros(n_tiles * P, np.int64)
    deg_desc[:n_nodes] = counts[nperm]

    batches = []            # (S, b, tile_start)
    t = 0
    while t < n_tiles:
        S = max(int(deg_desc[t * P]), 4)
        rem_groups = -(-(n_tiles - t) // N_CORES)
        b = max(1, min(MAX_B, MAX_BATCH_SLOTS // S, rem_groups))
        batches.append((S, b, t))
        t += N_CORES * b
    batches = batches[-1:] + batches[:-1]   # smallest batch first: fast ramp

    total_tiles = sum(N_CORES * b for S, b, _ in batches)
    rnode = np.full(total_tiles * P, -1, np.int64)
    rnode[:n_nodes] = nperm
    return dict(counts=counts, order_e=order_e, cum=cum, rnode=rnode,
                batches=batches, n_edges=n_edges, n_nodes=n_nodes)


def _prep_inputs(value, key, query_0, query_1, plan):
    import ml_dtypes
    f16 = np.float16
    f8 = ml_dtypes.float8_e3m4

    batches = plan["batches"]
    rnode = plan["rnode"]
    counts, order_e, cum = plan["counts"], plan["order_e"], plan["cum"]
    n_edges = plan["n_edges"]
    n_nodes = plan["n_nodes"]

    key_f = np.asarray(key, dtype=np.float32).reshape(n_edges, NF)
    val_f = np.asarray(value, dtype=np.float32).reshape(n_edges, NF)
    q_cat = np.concatenate(
        [np.asarray(query_0, np.float32), np.asarray(query_1, np.float32)],
        axis=-1).reshape(n_nodes, NF)
    # pad keys: every head must score far below real scores; with
    # k_pad = -K_PAD*sign(q) each head's pad score is -K_PAD*sum|q_j|
    kpad = (-K_PAD * np.sign(q_cat)).astype(np.float32)

    W = sum(S * b for S, b, _ in batches) * P
    Wq = sum(b for S, b, _ in batches) * P
    kT_all = np.empty((N_CORES, P, W), f8)
    v_all = np.empty((N_CORES, P, W), f16)
    qT_all = np.empty((N_CORES, P, Wq), f16)

    off = 0
    offq = 0
    for S, b, t0 in batches:
        nb = N_CORES * b * P
        rows = rnode[t0 * P:t0 * P + nb]
        valid_r = rows >= 0
        rr = np.where(valid_r, rows, 0)
        deg = np.where(valid_r, counts[rr], 0)
        start = cum[rr]
        sl = np.arange(S)
        eix = start[:, None] + sl[None, :]
        vmask = sl[None, :] < deg[:, None]
        eid = order_e[np.clip(eix, 0, n_edges - 1)]

        kg = key_f[eid]                        # [8*b*128, S, NF]
        kp = kpad[rr] * valid_r[:, None]
        kg = np.where(vmask[..., None], kg, kp[:, None, :])
        vg = val_f[eid]
        vg[~vmask] = 0.0
        qg = q_cat[rr] * valid_r[:, None]

        # [core, t, n, S, NF]
        kg = kg.reshape(N_CORES, b, P, S, NF)
        vg = vg.reshape(N_CORES, b, P, S, NF)
        qg = qg.reshape(N_CORES, b, P, NF)
        # kT: [f, (t, s, n)]
        kT_all[:, :, off:off + b * S * P] = kg.transpose(0, 4, 1, 3, 2).reshape(
            N_CORES, NF, b * S * P).astype(f8)
        # v: [n, (t, s, f)]
        v_all[:, :, off:off + b * S * P] = vg.transpose(0, 2, 1, 3, 4).reshape(
            N_CORES, P, b * S * NF).astype(f16)
        # qT: [f, (t, n)]
        qT_all[:, :, offq:offq + b * P] = qg.transpose(0, 3, 1, 2).reshape(
            N_CORES, NF, b * P).astype(f16)
        off += b * S * P
        offq += b * P

    hm = np.zeros((NF, H), np.float32)
    for h in range(H):
        hm[h * HS:(h + 1) * HS, h] = 1.0
    hm = hm.astype(f16)
    ident8 = np.eye(P, dtype=np.float32).astype(ml_dtypes.float8_e4m3)

    in_maps = []
    for c in range(N_CORES):
        in_maps.append({
            "kT": kT_all[c], "v": v_all[c], "qT": qT_all[c], "hm": hm,
            "ident8": ident8,
        })
    return in_maps


def _assemble(results, plan):
    batches = plan["batches"]
    n_nodes = plan["n_nodes"]
    rnode = plan["rnode"]
    out = np.zeros((n_nodes, NF), np.float32)
    for c in range(N_CORES):
        arr = np.asarray(results[c]["out"], dtype=np.float32)  # [P, Wq]
        offq = 0
        for S, b, t0 in batches:
            blk = arr[:, offq:offq + b * P].reshape(P, b, NF)
            rows = rnode[(t0 + c * b) * P:(t0 + (c + 1) * b) * P].reshape(
                b, P)
            for t in range(b):
                idx = rows[t]
                m = idx >= 0
                out[idx[m]] = blk[m, t]
            offq += b * P
    return out.reshape(n_nodes, NF // 4, 4)


def _get_nc(batch_prof):
    key = tuple(batch_prof)
    if key not in _CACHE:
        _CACHE[key] = build_nc(batch_prof)
    return _CACHE[key]


def _run(inputs, trace=False, **spmd_kwargs):
    global LAST_RESULTS
    from concourse.bass_utils import run_bass_kernel_spmd

    n_nodes = np.asarray(inputs["query_0"]).shape[0]
    plan = _plan(inputs["edge_index"], n_nodes)
    batch_prof = tuple((int(S), int(b)) for S, b, _ in plan["batches"])
    nc = _get_nc(batch_prof)
    in_maps = _prep_inputs(
        inputs["value"], inputs["key"], inputs["query_0"], inputs["query_1"],
        plan)
    res = run_bass_kernel_spmd(
        nc, in_maps, list(range(N_CORES)), trace=trace, **spmd_kwargs)
    LAST_RESULTS = res
    return _assemble(res.results, plan)


def kernel(value, key, query_0, query_1, edge_index):
    return _run({
        "value": value, "key": key, "query_0": query_0,
        "query_1": query_1, "edge_index": edge_index,
    })
